# revision 20
# baseline (speedup 1.0000x reference)
"""2-layer GAT on 8 Trainium2 NeuronCores (Bass/Tile, SPMD) — v2.

Strategy (edge-parallel, dst-sharded): edges are sharded by destination-node
range (6250 nodes / core) and, per core, sorted by (src<32768 split,
dst-block-of-128, src). Per 128-edge tile the kernel
  - SWDGE-gathers the 128 source rows (the ONLY per-edge DMA: 256B/edge),
  - builds a one-hot [edge x dst] matrix on DVE (is_equal vs an iota const),
  - gets per-edge dst attention values ad_e with a PE transpose + small
    matmul against a per-block [128 x H] table (no per-edge dst gather),
  - computes w = exp(leakyrelu(as_e + ad_e)) on DVE/ACT,
  - segment-sums w*h and w into PSUM via a PE matmul with the one-hot as
    stationary (replaces v1's dma_scatter_add RMW packets entirely).
Per-dst-block U/S accumulators live in SBUF; softmax is the U/S ratio so no
per-edge normalization. Layer boundaries fuse normalize+ELU+projection per
block. 4 SWDGE queues round-robin so descriptor generation overlaps DMA.
"""
import os
import numpy as np
import ml_dtypes

from concourse import bacc, mybir, tile
from concourse.bass_utils import run_bass_kernel_spmd

NCORES = 8
CH = 64          # feature channels in both layer tables (L2 zero-padded 40->64)
GC = int(os.environ.get("GAT_GC", "1024"))  # idxs per SWDGE gather call
TPG = GC // 128  # tiles per gather group
SPLIT = 32768    # int16 gather index reach (rows)
NQ = 4           # SWDGE queues, round-robin over gather calls
F32 = mybir.dt.float32
BF16 = mybir.dt.bfloat16
I16 = mybir.dt.int16
AL = mybir.AluOpType
AF = mybir.ActivationFunctionType

_prog_cache = {}
LAST_RESULTS = None  # BassKernelResults of the last device run (for test.py)


def _build(meta):
    NLOCP = meta["NLOCP"]
    NB = meta["NB"]
    NPAD = NCORES * NLOCP
    NGl, NGh = meta["NGl"], meta["NGh"]
    tiles_l, tiles_h = meta["tiles_l"], meta["tiles_h"]  # [(blk, first, last)]
    SL, SH = NGl * GC, NGh * GC

    nc = bacc.Bacc(num_devices=NCORES, num_swdge_queues=NQ,
                   dynamic_dma_scratch_size=16 * GC)

    # ---- I/O ----
    xTs = nc.dram_tensor("xTs", [128, NLOCP], F32, kind="ExternalInput")
    W1 = nc.dram_tensor("W1", [128, CH], F32, kind="ExternalInput")
    W2p = nc.dram_tensor("W2p", [CH, CH], F32, kind="ExternalInput")
    IOTAB = nc.dram_tensor("IOTAB", [128, 128], BF16, kind="ExternalInput")
    IOTAP = nc.dram_tensor("IOTAP", [128, 1], F32, kind="ExternalInput")
    ONEB = nc.dram_tensor("ONEB", [1, 128], BF16, kind="ExternalInput")
    IDN = nc.dram_tensor("IDN", [128, 128], F32, kind="ExternalInput")
    asrc1t = nc.dram_tensor("asrc1t", [128, CH], F32, kind="ExternalInput")
    adst1r = nc.dram_tensor("adst1r", [128, CH], F32, kind="ExternalInput")
    b1r = nc.dram_tensor("b1r", [128, CH], F32, kind="ExternalInput")
    b2r = nc.dram_tensor("b2r", [128, 40], F32, kind="ExternalInput")
    gl = nc.dram_tensor("gl", [128, SL // 16], I16, kind="ExternalInput")
    gh = nc.dram_tensor("gh", [128, SH // 16], I16, kind="ExternalInput")
    dTl = nc.dram_tensor("dTl", [128, SL // 128], BF16, kind="ExternalInput")
    dTh = nc.dram_tensor("dTh", [128, SH // 128], BF16, kind="ExternalInput")
    dRl = nc.dram_tensor("dRl", [1, SL], BF16, kind="ExternalInput")
    dRh = nc.dram_tensor("dRh", [1, SH], BF16, kind="ExternalInput")
    OUT = nc.dram_tensor("OUT", [NLOCP, 40], F32, kind="ExternalOutput")

    # ---- scratch ----
    h1loc = nc.dram_tensor("h1loc", [NLOCP, CH], F32, kind="Internal")
    h1full = nc.dram_tensor("h1full", [NPAD, CH], F32, kind="Internal",
                            addr_space="Shared")
    ab1 = nc.dram_tensor("ab1", [NLOCP, 4], F32, kind="Internal")
    h2loc = nc.dram_tensor("h2loc", [NLOCP, CH], F32, kind="Internal")
    h2full = nc.dram_tensor("h2full", [NPAD, CH], F32, kind="Internal",
                            addr_space="Shared")
    ab2 = nc.dram_tensor("ab2", [NLOCP, 1], F32, kind="Internal")

    debug = bool(os.environ.get("GAT_DEBUG"))
    if debug:
        Dh1 = nc.dram_tensor("Dh1", [NLOCP, CH], F32, kind="ExternalOutput")
        DU1 = nc.dram_tensor("DU1", [NLOCP, 68], F32, kind="ExternalOutput")
        Dh2 = nc.dram_tensor("Dh2", [NLOCP, CH], F32, kind="ExternalOutput")
        DU2 = nc.dram_tensor("DU2", [NLOCP, 41], F32, kind="ExternalOutput")

    groups = [list(range(NCORES))]

    with tile.TileContext(nc) as tc:
        with (
            tc.tile_pool(name="const", bufs=1) as cpool,
            tc.tile_pool(name="dense", bufs=3) as dpool,
            tc.tile_pool(name="dpsum", bufs=1, space="PSUM") as dps,
            tc.tile_pool(name="tpsum", bufs=2, space="PSUM") as tps,
            tc.tile_pool(name="apsum", bufs=2, space="PSUM") as aps,
            tc.tile_pool(name="upsum", bufs=2, space="PSUM") as ups,
            tc.tile_pool(name="idx", bufs=3) as ipool,
            tc.tile_pool(name="edge", bufs=3) as epool,
            tc.tile_pool(name="onehot", bufs=3) as opool,
            tc.tile_pool(name="msg", bufs=3) as mpool,
            tc.tile_pool(name="small", bufs=3) as spool,
            tc.tile_pool(name="ab", bufs=2) as abpool,
        ):
            # constants
            w1sb = cpool.tile([128, CH], F32)
            nc.sync.dma_start(w1sb[:], W1[:])
            w2sb = cpool.tile([CH, CH], F32)
            nc.sync.dma_start(w2sb[:], W2p[:])
            iotab = cpool.tile([128, 128], BF16)
            nc.sync.dma_start(iotab[:], IOTAB[:])
            iotap = cpool.tile([128, 1], F32)
            nc.sync.dma_start(iotap[:], IOTAP[:])
            oneb = cpool.tile([1, 128], BF16)
            nc.sync.dma_start(oneb[:], ONEB[:])
            idn = cpool.tile([128, 128], F32)
            nc.sync.dma_start(idn[:], IDN[:])
            as1sb = cpool.tile([128, CH], F32)
            nc.sync.dma_start(as1sb[:], asrc1t[:])
            ad1sb = cpool.tile([128, CH], F32)
            nc.sync.dma_start(ad1sb[:], adst1r[:])
            b1sb = cpool.tile([128, CH], F32)
            nc.sync.dma_start(b1sb[:], b1r[:])
            b2sb = cpool.tile([128, 40], F32)
            nc.sync.dma_start(b2sb[:], b2r[:])

            # persistent per-block U/S accumulators (SBUF)
            U1 = cpool.tile([128, NB, 68], F32)
            U2 = cpool.tile([128, NB, 41], F32)

            # ---- D1: h1 = x @ W1 (node slice); ab1 = per-head h1 . a_dst1 ----
            for i in range(NB):
                r0 = i * 128
                xt = dpool.tile([128, 128], F32, tag="xt")
                nc.sync.dma_start(xt[:], xTs[:, r0:r0 + 128])
                ps = dps.tile([128, CH], F32, tag="mm")
                nc.tensor.matmul(ps[:], xt[:], w1sb[:])
                ht = dpool.tile([128, CH], F32, tag="ht")
                nc.vector.tensor_copy(ht[:], ps[:])
                nc.sync.dma_start(h1loc[r0:r0 + 128, :], ht[:])
                tmp = dpool.tile([128, CH], F32, tag="tmp")
                nc.vector.tensor_mul(tmp[:], ht[:], ad1sb[:])
                dp = spool.tile([128, 4], F32, tag="dp")
                nc.vector.tensor_reduce(
                    dp[:], tmp[:].rearrange("p (h c) -> p h c", c=16),
                    mybir.AxisListType.X, AL.add)
                nc.sync.dma_start(ab1[r0:r0 + 128, :], dp[:])

            # ---- AllGather h1 ----
            nc.gpsimd.collective_compute(
                "AllGather", AL.bypass, groups, [h1loc[:, :]], [h1full[:, :]])

            def edge_pass(pass_id, tiles, NG, gidx, dT, dR, base, abt, U, H,
                          UW, as_fold, first_pass_for_blk, qoff):
                """One lo/hi pass over all dst blocks of one layer.
                tiles: [(blk, seg_first, seg_last)] per tile slot.
                base: gather base AP; abt: [NLOCP, H] per-dst attention table.
                U: [128, NB, UW] SBUF accumulator. as_fold: None (L1: compute
                as_e from gathered rows) or column index (L2)."""
                cur_ab = {}
                U_ps = None
                for g in range(NG):
                    it = ipool.tile([128, GC // 16], I16, tag="it")
                    nc.sync.dma_start(it[:], gidx[:, g * (GC // 16):(g + 1) * (GC // 16)])
                    dt = ipool.tile([128, TPG], BF16, tag="dt")
                    nc.sync.dma_start(dt[:], dT[:, g * TPG:(g + 1) * TPG])
                    dr = ipool.tile([1, GC], BF16, tag="dr")
                    nc.sync.dma_start(dr[:], dR[:, g * GC:(g + 1) * GC])
                    G = epool.tile([128, TPG, CH], F32, tag="G")
                    nc.gpsimd.dma_gather(
                        G[:], base, it[:], GC, GC, CH,
                        queue_num=(g + qoff) % NQ)

                    # one-hot [e, d] per tile (bf16; pads match nothing)
                    OH = opool.tile([128, TPG, 128], BF16, tag="OH")
                    nc.vector.tensor_tensor(
                        OH[:],
                        iotab[:].unsqueeze(1).broadcast_to([128, TPG, 128]),
                        dt[:].unsqueeze(2).broadcast_to([128, TPG, 128]),
                        AL.is_equal)

                    # OHT [d, e] via rank-1 outer (dloc bcast down partitions)
                    # + is_equal vs partition iota; ad_e = OHT^T @ ab_blk
                    ad_ps = aps.tile([128, TPG, H], F32, tag="ad")
                    for s in range(TPG // 4):
                        op4 = tps.tile([128, 4, 128], F32, tag="tp")
                        for t4 in range(4):
                            t = s * 4 + t4
                            nc.tensor.matmul(
                                op4[:, t4:t4 + 1, :].rearrange("p a b -> p (a b)"),
                                oneb[:],
                                dr[:, t * 128:(t + 1) * 128])
                        OHT = opool.tile([128, 4, 128], F32, tag="OHT")
                        nc.vector.tensor_tensor(
                            OHT[:], op4[:],
                            iotap[:].unsqueeze(2).broadcast_to([128, 4, 128]),
                            AL.is_equal)
                        for t4 in range(4):
                            t = s * 4 + t4
                            blk = tiles[g * TPG + t][0]
                            if blk not in cur_ab:
                                abk = abpool.tile([128, H], F32, tag="abk")
                                nc.sync.dma_start(
                                    abk[:], abt[blk * 128:(blk + 1) * 128, :])
                                cur_ab = {blk: abk}
                            nc.tensor.matmul(
                                ad_ps[:, t:t + 1, :].rearrange("p a b -> p (a b)"),
                                OHT[:, t4:t4 + 1, :].rearrange("p a b -> p (a b)"),
                                cur_ab[blk][:])

                    # as_e
                    if as_fold is None:
                        P = epool.tile([128, TPG, CH], F32, tag="P")
                        nc.vector.tensor_mul(
                            P[:], G[:],
                            as1sb[:].unsqueeze(1).broadcast_to([128, TPG, CH]))
                        asg = spool.tile([128, TPG, H], F32, tag="asg")
                        nc.vector.tensor_reduce(
                            asg[:],
                            P[:].rearrange("p t (h c) -> p t h c", c=CH // H),
                            mybir.AxisListType.X, AL.add)
                        as_ap = asg[:]
                    else:
                        as_ap = G[:, :, as_fold:as_fold + 1]

                    # w = exp(leakyrelu(as_e + ad_e))
                    e = spool.tile([128, TPG, H], F32, tag="e")
                    nc.vector.tensor_add(e[:], as_ap, ad_ps[:])
                    lr = spool.tile([128, TPG * H], F32, tag="lr")
                    nc.vector.scalar_tensor_tensor(
                        lr[:], e[:].rearrange("p a b -> p (a b)"), 0.2,
                        e[:].rearrange("p a b -> p (a b)"), AL.mult, AL.max)
                    w = spool.tile([128, TPG * H], F32, tag="w")
                    nc.scalar.activation(w[:], lr[:], AF.Exp)
                    wb = w[:].rearrange("p (t h) -> p t h", h=H)

                    # msg = [w*h | w] (bf16)
                    msg = mpool.tile([128, TPG, UW], BF16, tag="msg")
                    if H > 1:
                        nc.vector.tensor_mul(
                            msg[:, :, 0:CH].rearrange("p t (h c) -> p t h c", c=CH // H),
                            G[:].rearrange("p t (h c) -> p t h c", c=CH // H),
                            wb.unsqueeze(3).broadcast_to([128, TPG, H, CH // H]))
                        nc.vector.tensor_copy(msg[:, :, CH:CH + H], wb)
                    else:
                        nc.vector.tensor_mul(
                            msg[:, :, 0:UW - 1], G[:, :, 0:UW - 1],
                            wb.broadcast_to([128, TPG, UW - 1]))
                        nc.vector.tensor_copy(msg[:, :, UW - 1:UW], wb)

                    # segment-sum via PE: U_ps[d, :] += OH^T @ msg
                    for t in range(TPG):
                        blk, sfirst, slast = tiles[g * TPG + t]
                        if sfirst:
                            U_ps = ups.tile([128, UW], F32, tag="ups")
                        nc.tensor.matmul(
                            U_ps[:],
                            OH[:, t:t + 1, :].rearrange("p a b -> p (a b)"),
                            msg[:, t:t + 1, :].rearrange("p a b -> p (a b)"),
                            start=sfirst, stop=slast)
                        if slast:
                            if first_pass_for_blk[blk] == pass_id:
                                nc.vector.tensor_copy(U[:, blk, :], U_ps[:])
                            else:
                                nc.vector.tensor_add(
                                    U[:, blk, :], U[:, blk, :], U_ps[:])
                return NG

            # ---- E1 ----
            fp1 = meta["first_pass_for_blk"]
            edge_pass(0, tiles_l, NGl, gl, dTl, dRl, h1full[0:SPLIT, :], ab1,
                      U1, 4, 68, None, fp1, 0)
            edge_pass(1, tiles_h, NGh, gh, dTh, dRh, h1full[SPLIT:NPAD, :],
                      ab1, U1, 4, 68, None, fp1, NGl)

            # ---- F1: z = U/S + b1; ELU; h2 = mid @ W2p; ab2 ----
            for i in range(NB):
                r0 = i * 128
                ut = U1[:, i, :]
                sp = spool.tile([128, 4], F32, tag="sp")
                nc.vector.tensor_scalar(out=sp[:], in0=ut[:, 64:68],
                                        scalar1=1e-16, scalar2=None, op0=AL.add)
                rec = spool.tile([128, 4], F32, tag="rec")
                nc.vector.reciprocal(rec[:], sp[:])
                z = dpool.tile([128, CH], F32, tag="z")
                for h in range(4):
                    nc.vector.scalar_tensor_tensor(
                        z[:, 16 * h:16 * h + 16], ut[:, 16 * h:16 * h + 16],
                        rec[:, h:h + 1], b1sb[:, 16 * h:16 * h + 16],
                        AL.mult, AL.add)
                # ELU(z) = relu(z) + exp(min(z,0)) - 1
                r = dpool.tile([128, CH], F32, tag="r")
                nc.scalar.activation(r[:], z[:], AF.Relu)
                u = dpool.tile([128, CH], F32, tag="u")
                nc.vector.tensor_scalar(out=u[:], in0=z[:], scalar1=0.0,
                                        scalar2=None, op0=AL.min)
                tE = dpool.tile([128, CH], F32, tag="tE")
                nc.scalar.activation(tE[:], u[:], AF.Exp)
                mid = dpool.tile([128, CH], F32, tag="mid")
                nc.vector.scalar_tensor_tensor(
                    mid[:], tE[:], -1.0, r[:], AL.add, AL.add)
                tp2 = dps.tile([CH, 128], F32, tag="tp2")
                nc.tensor.transpose(tp2[:], mid[:, 0:CH], idn[:])
                tps2 = dpool.tile([CH, 128], F32, tag="tps2")
                nc.vector.tensor_copy(tps2[:], tp2[:])
                ps2 = dps.tile([128, CH], F32, tag="mm")
                nc.tensor.matmul(ps2[:], tps2[:], w2sb[:])
                h2t = dpool.tile([128, CH], F32, tag="h2t")
                nc.vector.tensor_copy(h2t[:], ps2[:])
                nc.sync.dma_start(h2loc[r0:r0 + 128, :], h2t[:])
                ab2t = spool.tile([128, 1], F32, tag="ab2t")
                nc.vector.tensor_copy(ab2t[:], h2t[:, 41:42])
                nc.sync.dma_start(ab2[r0:r0 + 128, :], ab2t[:])

            # ---- AllGather h2 ----
            nc.gpsimd.collective_compute(
                "AllGather", AL.bypass, groups, [h2loc[:, :]], [h2full[:, :]])

            # ---- E2 (same edge order; as/ad folded into table cols 40/41) ----
            edge_pass(0, tiles_l, NGl, gl, dTl, dRl, h2full[0:SPLIT, :], ab2,
                      U2, 1, 41, 40, fp1, 0)
            edge_pass(1, tiles_h, NGh, gh, dTh, dRh, h2full[SPLIT:NPAD, :],
                      ab2, U2, 1, 41, 40, fp1, NGl)

            # ---- F2: out = U2/S2 + b2 ----
            for i in range(NB):
                r0 = i * 128
                ut = U2[:, i, :]
                sp = spool.tile([128, 1], F32, tag="sp2")
                nc.vector.tensor_scalar(out=sp[:], in0=ut[:, 40:41],
                                        scalar1=1e-16, scalar2=None, op0=AL.add)
                rec = spool.tile([128, 1], F32, tag="rec2")
                nc.vector.reciprocal(rec[:], sp[:])
                ot = dpool.tile([128, 40], F32, tag="ot")
                nc.vector.scalar_tensor_tensor(
                    ot[:], ut[:, 0:40], rec[:, 0:1], b2sb[:], AL.mult, AL.add)
                nc.sync.dma_start(OUT[r0:r0 + 128, :], ot[:])

            if debug:
                for i in range(NB):
                    r0 = i * 128
                    for src_d, dst_d, w_ in ((h1loc, Dh1, CH), (h2loc, Dh2, CH)):
                        tt = dpool.tile([128, w_], F32, tag=f"dbg{w_}")
                        nc.sync.dma_start(tt[:], src_d[r0:r0 + 128, :])
                        nc.sync.dma_start(dst_d[r0:r0 + 128, :], tt[:])
                    du1 = dpool.tile([128, 68], F32, tag="du1")
                    nc.vector.tensor_copy(du1[:], U1[:, i, :])
                    nc.sync.dma_start(DU1[r0:r0 + 128, :], du1[:])
                    du2 = dpool.tile([128, 41], F32, tag="du2")
                    nc.vector.tensor_copy(du2[:], U2[:, i, :])
                    nc.sync.dma_start(DU2[r0:r0 + 128, :], du2[:])

    nc.finalize()
    return nc


def _wrap_idx(a):
    """int16 [cap] -> wrapped [16, cap/16] replicated to [128, cap/16]."""
    w = a.reshape(-1, 16).T.copy()
    return np.ascontiguousarray(np.tile(w, (8, 1)))


def prep(x, edge_index, W1, a_src1, a_dst1, b1, W2, a_src2, a_dst2, b2):
    """Host-side sharding/index prep. Returns (meta, in_maps, (N, FIN))."""
    x = np.asarray(x, np.float32)
    N, FIN = x.shape
    NLOC = (N + NCORES - 1) // NCORES                       # 6250
    NLOCP = ((NLOC + 127) // 128) * 128                     # 6272
    NB = NLOCP // 128                                       # 49
    NPAD = NCORES * NLOCP

    ei0 = np.asarray(edge_index[0]).astype(np.int64)
    ei1 = np.asarray(edge_index[1]).astype(np.int64)
    loops = np.arange(N, dtype=np.int64)
    src = np.concatenate([ei0, loops])
    dst = np.concatenate([ei1, loops])

    rmap = (src // NLOC) * NLOCP + (src % NLOC)             # table row of src
    core = dst // NLOC
    dloc = dst % NLOC

    # per (core, pass): edges sorted by (dst-block, src-row)
    per = []  # [core][pass] = (rs_sorted, blk_sorted, dlm_sorted)
    for c in range(NCORES):
        m = core == c
        rs_c, dl_c = rmap[m], dloc[m]
        blk_c, dlm_c = dl_c // 128, dl_c % 128
        lo = rs_c < SPLIT
        rows = []
        for p, sel in enumerate((lo, ~lo)):
            rs, blk, dlm = rs_c[sel], blk_c[sel], dlm_c[sel]
            o = np.lexsort((rs, blk))
            rows.append((rs[o] - (SPLIT if p else 0), blk[o], dlm[o]))
        per.append(rows)

    # uniform tiles-per-(pass, block) across cores
    TPB = np.zeros((2, NB), np.int64)
    for c in range(NCORES):
        for p in range(2):
            cnt = np.bincount(per[c][p][1], minlength=NB)
            TPB[p] = np.maximum(TPB[p], (cnt + 127) // 128)
    first_pass_for_blk = np.where(TPB[0] > 0, 0, 1).tolist()

    def pass_meta(p):
        bids = np.repeat(np.arange(NB), TPB[p])
        NG = (len(bids) + TPG - 1) // TPG
        pad = NG * TPG - len(bids)
        if pad:
            bids = np.concatenate([bids, np.full(pad, bids[-1])])
        first = np.ones(len(bids), bool)
        first[1:] = bids[1:] != bids[:-1]
        last = np.ones(len(bids), bool)
        last[:-1] = bids[1:] != bids[:-1]
        return NG, list(zip(bids.tolist(), first.tolist(), last.tolist()))

    NGl, tiles_l = pass_meta(0)
    NGh, tiles_h = pass_meta(1)
    slot_base = [np.concatenate([[0], np.cumsum(TPB[p]) * 128]) for p in range(2)]

    # ---- constant inputs (replicated) ----
    W1 = np.asarray(W1, np.float32)
    W2p = np.zeros((CH, CH), np.float32)
    W2p[:, :40] = np.asarray(W2, np.float32)
    W2p[:, 40] = W2p[:, :40] @ np.asarray(a_src2, np.float32).reshape(40)
    W2p[:, 41] = W2p[:, :40] @ np.asarray(a_dst2, np.float32).reshape(40)
    IOTAB = np.ascontiguousarray(np.tile(
        np.arange(128, dtype=ml_dtypes.bfloat16)[None, :], (128, 1)))
    IOTAP = np.arange(128, dtype=np.float32)[:, None].copy()
    ONEB = np.ones((1, 128), dtype=ml_dtypes.bfloat16)
    IDN = np.eye(128, dtype=np.float32)
    as1 = np.asarray(a_src1, np.float32).reshape(CH)
    ad1 = np.asarray(a_dst1, np.float32).reshape(CH)
    asrc1t = np.ascontiguousarray(np.tile(as1[None, :], (128, 1)))
    adst1r = np.ascontiguousarray(np.tile(ad1[None, :], (128, 1)))
    b1r = np.ascontiguousarray(
        np.tile(np.asarray(b1, np.float32)[None, :], (128, 1)))
    b2r = np.ascontiguousarray(
        np.tile(np.asarray(b2, np.float32)[None, :], (128, 1)))

    xpad = np.zeros((NPAD, FIN), np.float32)
    for c in range(NCORES):
        n0 = c * NLOC
        take = min(NLOCP, N - n0)
        xpad[c * NLOCP:c * NLOCP + take] = x[n0:n0 + take]

    in_maps = []
    for c in range(NCORES):
        packs = []
        for p, NG in ((0, NGl), (1, NGh)):
            slots = NG * GC
            idx_arr = np.zeros(slots, np.int64)
            dloc_arr = np.full(slots, 999.0, np.float32)
            rs, blk, dlm = per[c][p]
            if len(blk):
                starts = np.concatenate([[0], np.cumsum(np.bincount(blk, minlength=NB))])
                rank = np.arange(len(blk)) - starts[blk]
                pos = slot_base[p][blk] + rank
                idx_arr[pos] = rs
                dloc_arr[pos] = dlm.astype(np.float32)
            packs.append((
                _wrap_idx(idx_arr.astype(np.int16)),
                np.ascontiguousarray(
                    dloc_arr.reshape(-1, 128).T).astype(ml_dtypes.bfloat16),
                dloc_arr[None, :].astype(ml_dtypes.bfloat16)))
        xT = np.ascontiguousarray(xpad[c * NLOCP:(c + 1) * NLOCP].T)
        in_maps.append({
            "xTs": xT, "W1": W1, "W2p": W2p, "IOTAB": IOTAB, "IOTAP": IOTAP,
            "ONEB": ONEB, "IDN": IDN,
            "asrc1t": asrc1t, "adst1r": adst1r, "b1r": b1r, "b2r": b2r,
            "gl": packs[0][0], "dTl": packs[0][1], "dRl": packs[0][2],
            "gh": packs[1][0], "dTh": packs[1][1], "dRh": packs[1][2],
        })

    meta = {
        "NLOC": NLOC, "NLOCP": NLOCP, "NB": NB,
        "NGl": NGl, "NGh": NGh, "tiles_l": tiles_l, "tiles_h": tiles_h,
        "first_pass_for_blk": first_pass_for_blk,
    }
    return meta, in_maps, (N, FIN)


def kernel(**inputs):
    global LAST_RESULTS
    meta, in_maps, (N, FIN) = prep(**inputs)
    NLOC = meta["NLOC"]
    key = (N, FIN, meta["NGl"], meta["NGh"],
           tuple(t[0] for t in meta["tiles_l"]),
           tuple(t[0] for t in meta["tiles_h"]))
    if key not in _prog_cache:
        _prog_cache[key] = _build(meta)
    nc = _prog_cache[key]

    want_trace = bool(os.environ.get("GAT_TRACE"))
    if want_trace:
        try:
            from antenv import axon_hooks  # noqa: F401
        except ImportError:
            want_trace = False
    res = run_bass_kernel_spmd(
        nc, in_maps, core_ids=list(range(NCORES)), trace=want_trace)
    LAST_RESULTS = res
    out = np.empty((N, 40), np.float32)
    for c in range(NCORES):
        n0 = c * NLOC
        take = min(NLOC, N - n0)
        out[n0:n0 + take] = res.results[c]["OUT"][:take]
    return out
